# revision 2
# baseline (speedup 1.0000x reference)
"""Trainium2 Bass kernel for nn_DCM (dynamic conv module), data-parallel over
batch N=8 across 8 NeuronCores (1 sample per core).

Per-core program (sample n):
  x [512, 3600] fp32 -> cast to bf16 in SBUF
  for k in (1,3,5):
    f_k = relu(w1k' @ x + b1k)          (1x1 conv, BN scale folded into w)
    pooled_k = block-sums of x          (hierarchical DVE reductions, 1/area
                                         folded into w2)
    g_k = relu(w2k'' @ pooled_k + b2k)  (tiny matmul)
    o_k = relu(depthwise(f_k, g_k))     (k^2 diag(g) matmuls on shifted
                                         zero-padded windows, PSUM accum;
                                         k=1 is a fused scale+relu on ACT)
    d_k = relu(wfk' @ o_k + bfk)
  y = relu(w_out' @ [x;d1;d3;d5] + b_out)  (16 K-tiles accumulated in PSUM)

All matmuls bf16 (fp32 PSUM accumulate). Weights are pre-transposed,
BN-folded and bf16-cast on the host.
"""

import json

import numpy as np
import ml_dtypes

import concourse.bass as bass
import concourse.tile as tile
from concourse import mybir
from concourse.vector_clock import ScopedClock

P = 128
C = 512
C4 = 128
H = W = 60
HW = H * W
NB = 10          # bands
BR = 6           # rows per band
NT = BR * W      # 360 columns per band
N_CORES = 8
F32 = mybir.dt.float32
BF16 = mybir.dt.bfloat16
RELU = mybir.ActivationFunctionType.Relu

# ---------------------------------------------------------------------------
# Patches for walrus/concourse skew in this container: this walrus build only
# encodes ONE sync wait per instruction, while Tile emits several.
# 1) TileContext tail drain: emit its waits as 1-wait NOPs on SP instead.
# 2) to_json_bytes post-pass: split any instruction with N>1 waits into N-1
#    preceding same-engine 1-wait NOPs (same-engine program order makes this
#    semantically identical).
# ---------------------------------------------------------------------------


def _patched_drain_and_barrier(self, tick_clock, wait_clock):
    nc = self.nc
    probe = nc.sync.nop(nofuse=True)
    wait_clock.add_sem_waits(probe.ins, ScopedClock({None: tick_clock.global_clock}))
    si = probe.ins.sync_info
    waits = list(si.on_wait) if si is not None else []
    if len(waits) > 1:
        probe.ins.sync_info = mybir.SyncInfo(
            on_wait=[waits[0]], on_update=list(si.on_update)
        )
        for w in waits[1:]:
            n = nc.sync.nop(nofuse=True)
            n.ins.sync_info = mybir.SyncInfo(on_wait=[w], on_update=[])
    nc.sync.drain()
    nc.all_engine_barrier()
    assert self.sems is not None
    popped = nc._tile_sem_poison_stack.pop()
    assert popped is self._sem_poison
    nc.clear_and_free_semaphores(list(self.sems.allocated().values()))
    nc.all_engine_barrier()


def _split_waits_json(raw: bytes) -> bytes:
    m = json.loads(raw)
    ctr = 0
    changed = False
    for f in m.get("functions", []):
        for bb in f.get("blocks", []):
            out = []
            for inst in bb.get("instructions", []):
                si = inst.get("sync_info")
                waits = (si or {}).get("on_wait") or []
                if len(waits) > 1:
                    changed = True
                    for w in waits[:-1]:
                        ctr += 1
                        nop = {
                            "engine": inst.get("engine"),
                            "ins": [],
                            "outs": [],
                            "name": f"{inst['name']}-sw{ctr}",
                            "opcode": "NoOp",
                            "sync_info": {"on_update": [], "on_wait": [w]},
                        }
                        if "debug" in inst:
                            nop["debug"] = inst["debug"]
                        out.append(nop)
                    si["on_wait"] = [waits[-1]]
                out.append(inst)
            bb["instructions"] = out
    return json.dumps(m).encode() if changed else raw


_PATCHED = False


def _apply_patches():
    global _PATCHED
    if _PATCHED:
        return
    tile.TileContext._drain_and_barrier = _patched_drain_and_barrier
    orig = bass.Bass.to_json_bytes

    def _patched_to_json_bytes(self, *a, **kw):
        return _split_waits_json(orig(self, *a, **kw))

    bass.Bass.to_json_bytes = _patched_to_json_bytes
    _PATCHED = True


# ---------------------------------------------------------------------------
# Bass program
# ---------------------------------------------------------------------------


def _transposed_out_ap(t, inner_count, inner_stride, outer_count, outer_stride):
    """Out AP for a reduce whose iteration order is (inner, outer) but whose
    memory layout should be [outer][inner]."""
    return bass.AP(
        tensor=t.tensor,
        offset=t.offset,
        ap=[t.ap[0], [inner_stride, inner_count], [outer_stride, outer_count]],
    )


def _build_bass():
    _apply_patches()
    nc = bass.Bass(trn_type="TRN2")

    x_d = nc.dram_tensor("x", [C, HW], F32, kind="ExternalInput")
    w1T_d = nc.dram_tensor("w1T", [3, C, C4], BF16, kind="ExternalInput")
    b1_d = nc.dram_tensor("b1", [3, C4], F32, kind="ExternalInput")
    w2T_d = nc.dram_tensor("w2T", [3, C, C4], BF16, kind="ExternalInput")
    b2_d = nc.dram_tensor("b2", [3, C4], F32, kind="ExternalInput")
    wfT_d = nc.dram_tensor("wfT", [3, C4, C], BF16, kind="ExternalInput")
    bf_d = nc.dram_tensor("bf", [3, 4, P], F32, kind="ExternalInput")
    woT_d = nc.dram_tensor("woT", [4 * C, C], BF16, kind="ExternalInput")
    bo_d = nc.dram_tensor("bo", [4, P], F32, kind="ExternalInput")
    id_d = nc.dram_tensor("ident", [P, P], F32, kind="ExternalInput")
    y_d = nc.dram_tensor("y", [C, HW], F32, kind="ExternalOutput")

    with tile.TileContext(nc) as tc:
        with (
            tc.tile_pool(name="consts", bufs=1) as consts,
            tc.tile_pool(name="xpool", bufs=1) as xpool,
            tc.tile_pool(name="fpool", bufs=1) as fpool,
            tc.tile_pool(name="ptmp", bufs=2) as ptmp,
            tc.tile_pool(name="gpool", bufs=1) as gpool,
            tc.tile_pool(name="obuf", bufs=3) as obuf,
            tc.tile_pool(name="dbuf", bufs=3) as dbuf,
            tc.tile_pool(name="ybuf", bufs=3) as ybuf,
            tc.tile_pool(name="psum", bufs=4, space="PSUM") as psum,
        ):
            # ---- weights / constants -> SBUF ----
            w1T = consts.tile([P, 3, 4, C4], BF16)
            nc.sync.dma_start(w1T[:], w1T_d.rearrange("k (kt p) m -> p k kt m", p=P))
            w2T = consts.tile([P, 3, 4, C4], BF16)
            nc.sync.dma_start(w2T[:], w2T_d.rearrange("k (kt p) m -> p k kt m", p=P))
            wfT = consts.tile([P, 3, C], BF16)
            nc.sync.dma_start(wfT[:], wfT_d.rearrange("k p m -> p k m"))
            woT = consts.tile([P, 16, C], BF16)
            nc.sync.dma_start(woT[:], woT_d.rearrange("(kt p) m -> p kt m", p=P))
            b1 = consts.tile([P, 3], F32)
            nc.sync.dma_start(b1[:], b1_d.rearrange("k p -> p k"))
            b2 = consts.tile([P, 3], F32)
            nc.sync.dma_start(b2[:], b2_d.rearrange("k p -> p k"))
            bfb = consts.tile([P, 3, 4], F32)
            nc.sync.dma_start(bfb[:], bf_d.rearrange("k m p -> p k m"))
            bo = consts.tile([P, 4], F32)
            nc.sync.dma_start(bo[:], bo_d.rearrange("m p -> p m"))
            ident = consts.tile([P, P], F32)
            nc.sync.dma_start(ident[:], id_d[:])

            # ---- x -> bf16 SBUF (cast in SWDGE DMA), chunked per band ----
            x_sb = xpool.tile([P, 4, HW], BF16)
            for kt in range(4):
                for b in range(NB):
                    nc.gpsimd.dma_start(
                        x_sb[:, kt, b * NT:(b + 1) * NT],
                        x_d[kt * P:(kt + 1) * P, b * NT:(b + 1) * NT],
                    )

            # ---- f convs (k=1 plain, k=3/5 zero-padded layouts) ----
            f1 = fpool.tile([P, HW], BF16)
            f3 = fpool.tile([P, 64, 64], BF16)
            f5 = fpool.tile([P, 64, 64], BF16)
            nc.vector.memset(f3[:], 0.0)
            nc.vector.memset(f5[:], 0.0)
            for ki, fdst in ((0, f1), (1, f3), (2, f5)):
                for b in range(NB):
                    ps = psum.tile([P, NT], F32, tag="work")
                    for kt in range(4):
                        nc.tensor.matmul(
                            ps[:],
                            w1T[:, ki, kt, :],
                            x_sb[:, kt, b * NT:(b + 1) * NT],
                            start=(kt == 0),
                            stop=(kt == 3),
                        )
                    if ki == 0:
                        dst = fdst[:, b * NT:(b + 1) * NT]
                    else:
                        dst = fdst[:, 2 + b * BR: 2 + (b + 1) * BR, 2:62]
                    nc.scalar.activation(dst, ps[:], RELU,
                                         bias=b1[:, ki:ki + 1], scale=1.0)

            # ---- pooling: block sums via hierarchical 4x4 grid (DVE) ----
            pooled = {k: gpool.tile([P, 4, k * k], BF16, name=f"pooled{k}")
                      for k in (1, 3, 5)}
            for kt in range(4):
                xk = x_sb[:, kt, :]
                r1 = ptmp.tile([P, 15, 60], F32, tag="r1")  # [wb][h]
                nc.vector.reduce_sum(
                    _transposed_out_ap(r1, 60, 1, 15, 60),
                    xk.rearrange("p (h wb w) -> p h wb w", h=60, wb=15),
                    axis=mybir.AxisListType.X)
                q = ptmp.tile([P, 15, 15], F32, tag="q")  # [wb][hb]
                nc.vector.reduce_sum(
                    q[:], r1.rearrange("p wb (hb h) -> p wb hb h", hb=15),
                    axis=mybir.AxisListType.X)
                # k=5: 3x3 q-cells per block
                t5 = ptmp.tile([P, 5, 15], F32, tag="t5")  # [hbB][wb]
                nc.vector.reduce_sum(
                    _transposed_out_ap(t5, 15, 1, 5, 15),
                    q.rearrange("p wb (hbB hb) -> p wb hbB hb", hbB=5),
                    axis=mybir.AxisListType.X)
                p5 = ptmp.tile([P, 5, 5], F32, tag="p5")  # [i][j]
                nc.vector.reduce_sum(
                    p5[:], t5.rearrange("p hbB (wbB wb) -> p hbB wbB wb", wbB=5),
                    axis=mybir.AxisListType.X)
                nc.vector.tensor_copy(out=pooled[5][:, kt, :],
                                      in_=p5.rearrange("p a b -> p (a b)"))
                # k=3: 5x5 q-cells per block
                t3 = ptmp.tile([P, 3, 15], F32, tag="t3")  # [hbB][wb]
                nc.vector.reduce_sum(
                    _transposed_out_ap(t3, 15, 1, 3, 15),
                    q.rearrange("p wb (hbB hb) -> p wb hbB hb", hbB=3),
                    axis=mybir.AxisListType.X)
                p3 = ptmp.tile([P, 3, 3], F32, tag="p3")
                nc.vector.reduce_sum(
                    p3[:], t3.rearrange("p hbB (wbB wb) -> p hbB wbB wb", wbB=3),
                    axis=mybir.AxisListType.X)
                nc.vector.tensor_copy(out=pooled[3][:, kt, :],
                                      in_=p3.rearrange("p a b -> p (a b)"))
                # k=1: total sum
                p1 = ptmp.tile([P, 1], F32, tag="p1")
                nc.vector.reduce_sum(p1[:], q.rearrange("p a b -> p (a b)"),
                                     axis=mybir.AxisListType.X)
                nc.vector.tensor_copy(out=pooled[1][:, kt, :], in_=p1[:])

            # ---- g convs + diag builds ----
            g_sb = {}
            for ki, k in enumerate((1, 3, 5)):
                gp = psum.tile([P, k * k], F32, tag="work")
                for kt in range(4):
                    nc.tensor.matmul(gp[:], w2T[:, ki, kt, :], pooled[k][:, kt, :],
                                     start=(kt == 0), stop=(kt == 3))
                g = gpool.tile([P, k * k], F32, name=f"g{k}")
                nc.scalar.activation(g[:], gp[:], RELU,
                                     bias=b2[:, ki:ki + 1], scale=1.0)
                g_sb[k] = g
            diag = {}
            for k in (3, 5):
                dg = gpool.tile([P, k * k, P], BF16, name=f"diag{k}")
                for t in range(k * k):
                    nc.vector.tensor_scalar_mul(dg[:, t, :], ident[:],
                                                g_sb[k][:, t:t + 1])
                diag[k] = dg

            # ---- band loop ----
            for b in range(NB):
                # depthwise taps (k=3, k=5) accumulate in PSUM
                o_sb = {}
                for k, fpad in ((3, f3), (5, f5)):
                    pad = (k - 1) // 2
                    ps = psum.tile([P, NT], F32, tag="work")
                    t = 0
                    for i in range(k):
                        for j in range(k):
                            r0 = 2 + b * BR + i - pad
                            c0 = 2 + j - pad
                            nc.tensor.matmul(
                                ps[:], diag[k][:, t, :],
                                fpad[:, r0:r0 + BR, c0:c0 + W],
                                start=(t == 0), stop=(t == k * k - 1))
                            t += 1
                    o = obuf.tile([P, NT], BF16, tag=f"o{k}")
                    nc.scalar.activation(o[:], ps[:], RELU, bias=0.0, scale=1.0)
                    o_sb[k] = o
                # k=1: o1 = relu(g1 * f1)
                o1 = obuf.tile([P, NT], BF16, tag="o1")
                nc.scalar.activation(o1[:], f1[:, b * NT:(b + 1) * NT], RELU,
                                     bias=0.0, scale=g_sb[1][:, 0:1])
                o_sb[1] = o1

                # final conv accumulation: parts [x, d1, d3, d5]
                op = [psum.tile([P, NT], F32, tag="out", name=f"op{b}_{m}")
                      for m in range(4)]
                for m in range(4):
                    for kt in range(4):
                        nc.tensor.matmul(
                            op[m], woT[:, kt, m * P:(m + 1) * P],
                            x_sb[:, kt, b * NT:(b + 1) * NT],
                            start=(kt == 0), stop=False)
                for pi, k in enumerate((1, 3, 5)):
                    d_sb = dbuf.tile([P, 4, NT], BF16, tag="d")
                    for m in range(4):
                        dps = psum.tile([P, NT], F32, tag="work")
                        nc.tensor.matmul(dps[:], wfT[:, pi, m * P:(m + 1) * P],
                                         o_sb[k][:], start=True, stop=True)
                        # relu(x + bias) on DVE: (in + bf) max 0
                        nc.vector.tensor_scalar(
                            d_sb[:, m, :], dps[:],
                            bfb[:, pi, m:m + 1], 0.0,
                            op0=mybir.AluOpType.add, op1=mybir.AluOpType.max)
                    for m in range(4):
                        for ktl in range(4):
                            kt = 4 * (pi + 1) + ktl
                            nc.tensor.matmul(
                                op[m], woT[:, kt, m * P:(m + 1) * P],
                                d_sb[:, ktl, :],
                                start=False, stop=(pi == 2 and ktl == 3))
                # epilogue + store
                ysb = ybuf.tile([P, 4, NT], F32, tag="y")
                for m in range(4):
                    nc.scalar.activation(ysb[:, m, :], op[m], RELU,
                                         bias=bo[:, m:m + 1], scale=1.0)
                    nc.sync.dma_start(
                        y_d[m * P:(m + 1) * P, b * NT:(b + 1) * NT],
                        ysb[:, m, :])
    return nc


# ---------------------------------------------------------------------------
# Host side
# ---------------------------------------------------------------------------

_NC_CACHE = {}


def _get_nc():
    if "nc" not in _NC_CACHE:
        _NC_CACHE["nc"] = _build_bass()
    return _NC_CACHE["nc"]


def _host_prep(inputs):
    """Fold BN scales into weights, transpose into lhsT layouts, cast bf16."""
    bf16 = ml_dtypes.bfloat16
    f32 = np.float32

    def A(name):
        return np.asarray(inputs[name], f32)

    w1T = np.stack([(A(f"s1_{k}")[:, None] * A(f"w1_{k}")).T for k in (1, 3, 5)])
    b1 = np.stack([A(f"b1_{k}") for k in (1, 3, 5)])
    areas = {1: 3600.0, 3: 400.0, 5: 144.0}
    w2T = np.stack([((A(f"s2_{k}")[:, None] * A(f"w2_{k}")) / areas[k]).T
                    for k in (1, 3, 5)])
    b2 = np.stack([A(f"b2_{k}") for k in (1, 3, 5)])
    wfT = np.stack([(A(f"sf_{k}")[:, None] * A(f"wf_{k}")).T for k in (1, 3, 5)])
    bf = np.stack([A(f"bf_{k}").reshape(4, P) for k in (1, 3, 5)])
    woT = (A("s_out")[:, None] * A("w_out")).T
    bo = A("b_out").reshape(4, P)
    return {
        "w1T": np.ascontiguousarray(w1T).astype(bf16),
        "b1": np.ascontiguousarray(b1),
        "w2T": np.ascontiguousarray(w2T).astype(bf16),
        "b2": np.ascontiguousarray(b2),
        "wfT": np.ascontiguousarray(wfT).astype(bf16),
        "bf": np.ascontiguousarray(bf),
        "woT": np.ascontiguousarray(woT).astype(bf16),
        "bo": np.ascontiguousarray(bo),
        "ident": np.eye(P, dtype=f32),
    }


def _run(inputs, **kwargs):
    from concourse.bass_utils import run_bass_kernel_spmd

    common = _host_prep(inputs)
    x = np.asarray(inputs["x"], np.float32).reshape(N_CORES, C, HW)
    in_maps = [{**common, "x": np.ascontiguousarray(x[n])} for n in range(N_CORES)]
    return run_bass_kernel_spmd(_get_nc(), in_maps,
                                core_ids=list(range(N_CORES)), **kwargs)


def kernel(**inputs):
    res = _run(inputs)
    return np.stack([r["y"].reshape(C, H, W) for r in res.results]).astype(np.float32)
